# revision 1
# baseline (speedup 1.0000x reference)
"""A2C2f (area-attention C2f block) on 8 NeuronCores.

Sharding: data-parallel over (batch, row-half): 8 shards = 4 images x 2
halves of 24 rows. Attention is independent per 12-row area, so each
half (2 areas) is attention-local. The only cross-half coupling is the
7x7 depthwise position-encoding conv (3-row halo); each shard receives
a 36-row slab (its 2 areas + the adjacent area) and recomputes the halo
locally, so shards are fully independent (no collectives).

Validity of the halo recompute (top half, owned rows 0..23):
  - final out rows 0..23 need block-2 attention on areas 0,1 (rows
    0..23) and block-2 dwconv on v2 rows -3..26 -> y1 rows 0..26.
  - y1 rows 0..26 need block-1 attention on areas 0,1,2 (rows 0..35)
    and block-1 dwconv rows 0..26 -> v1 rows 0..29, all inside the
    36-row slab and unaffected by the fake zero-pad at slab row 35
    (only dwconv rows >= 33 see it).  Bottom half is symmetric.
"""

import numpy as np
import jax
import jax.numpy as jnp
from jax import lax

NUM_HEADS = 8
C1 = 512
CH = 256
HD = CH // NUM_HEADS
W48 = 48
SLAB = 36   # slab rows per shard (3 areas)
OWN = 24    # owned rows per shard (2 areas)
VEXT = 27   # rows of v needed for block-2 dwconv


def _conv1x1(x, w, b, act=False):
    y = jnp.einsum('bchw,oc->bohw', x, w[:, :, 0, 0]) + b[None, :, None, None]
    return jax.nn.silu(y) if act else y


def _dwconv7(x, w, b):
    y = lax.conv_general_dilated(x, w, (1, 1), ((3, 3), (3, 3)),
                                 dimension_numbers=('NCHW', 'OIHW', 'NCHW'),
                                 feature_group_count=x.shape[1])
    return y + b[None, :, None, None]


def _attn_from_qkv(qkv, n_areas):
    # qkv: (1, 3*CH, R, 48) with R = 12*n_areas; follows reference aattn
    B, _, R, W = qkv.shape
    N = R * W
    q3 = qkv.reshape(B, 3 * CH, N).transpose(0, 2, 1)
    q3 = q3.reshape(B * n_areas, N // n_areas, 3 * CH)
    Ba, Na = q3.shape[0], q3.shape[1]
    q3 = q3.reshape(Ba, Na, NUM_HEADS, 3 * HD).transpose(0, 2, 3, 1)
    q, k, v = q3[:, :, :HD], q3[:, :, HD:2 * HD], q3[:, :, 2 * HD:]
    attn = jax.nn.softmax(jnp.einsum('bhdn,bhdm->bhnm', q, k) * HD ** -0.5,
                          axis=-1)
    out = jnp.einsum('bhdm,bhnm->bhdn', v, attn)
    out = out.transpose(0, 3, 1, 2).reshape(B, R, W, CH).transpose(0, 3, 1, 2)
    return out


def _v_from_qkv(qkv):
    # per-pixel v in (h*32+d) channel order, matching reference's vv
    B, _, R, W = qkv.shape
    t = qkv.reshape(B, NUM_HEADS, 3 * HD, R, W)
    return t[:, :, 2 * HD:].reshape(B, CH, R, W)


def _ablock_full(x, wqkv, bqkv, wproj, bproj, wpe, bpe, wm1, bm1, wm2, bm2,
                 n_areas):
    qkv = _conv1x1(x, wqkv, bqkv)
    out = _attn_from_qkv(qkv, n_areas)
    vv = _v_from_qkv(qkv)
    x = x + _conv1x1(out + _dwconv7(vv, wpe, bpe), wproj, bproj)
    h = _conv1x1(x, wm1, bm1, act=True)
    return x + _conv1x1(h, wm2, bm2)


def _shard_fn(x_slab, x_own, s_attn, s_vext, s_dwout,
              w_cv1, b_cv1, w_qkv, b_qkv, w_projA, b_projA, w_pe, b_pe,
              w_mlp1, b_mlp1, w_mlp2, b_mlp2, w_cv2, b_cv2, gamma):
    # block 1 on the full 36-row slab (3 areas)
    y0 = _conv1x1(x_slab, w_cv1, b_cv1, act=True)
    y1 = _ablock_full(y0, w_qkv[0], b_qkv[0], w_projA[0], b_projA[0],
                      w_pe[0], b_pe[0], w_mlp1[0], b_mlp1[0],
                      w_mlp2[0], b_mlp2[0], n_areas=3)

    # block 2 on the owned 24 rows, dwconv halo from 27 rows of y1
    y1_ext = lax.dynamic_slice(y1, (0, 0, s_vext, 0), (1, CH, VEXT, W48))
    qkv2_ext = _conv1x1(y1_ext, w_qkv[1], b_qkv[1])
    qkv2_own = lax.dynamic_slice(
        qkv2_ext, (0, 0, s_attn - s_vext, 0), (1, 3 * CH, OWN, W48))
    attn2 = _attn_from_qkv(qkv2_own, n_areas=2)
    vv2 = _v_from_qkv(qkv2_ext)
    pe2 = _dwconv7(vv2, w_pe[1], b_pe[1])
    pe2 = lax.dynamic_slice(pe2, (0, 0, s_dwout, 0), (1, CH, OWN, W48))
    ya = lax.dynamic_slice(y1, (0, 0, s_attn, 0), (1, CH, OWN, W48))
    x2 = ya + _conv1x1(attn2 + pe2, w_projA[1], b_projA[1])
    h2 = _conv1x1(x2, w_mlp1[1], b_mlp1[1], act=True)
    y2 = x2 + _conv1x1(h2, w_mlp2[1], b_mlp2[1])

    y0_own = lax.dynamic_slice(y0, (0, 0, s_attn, 0), (1, CH, OWN, W48))
    cat = jnp.concatenate([y0_own, y2], axis=1)
    out = _conv1x1(cat, w_cv2, b_cv2, act=True)
    return x_own + gamma[None, :, None, None] * out


_WNAMES = ['w_cv1', 'b_cv1', 'w_qkv', 'b_qkv', 'w_projA', 'b_projA',
           'w_pe', 'b_pe', 'w_mlp1', 'b_mlp1', 'w_mlp2', 'b_mlp2',
           'w_cv2', 'b_cv2', 'gamma']

_pmapped = None


def _get_pmap(n_dev):
    global _pmapped
    if _pmapped is None:
        _pmapped = jax.pmap(
            _shard_fn,
            in_axes=(0, 0, 0, 0, 0) + (None,) * len(_WNAMES),
            devices=jax.devices()[:n_dev])
    return _pmapped


def kernel(**inputs):
    x = np.asarray(inputs['x'])
    B = x.shape[0]
    n_shards = 2 * B  # 8

    x_slabs = np.empty((n_shards, 1, C1, SLAB, W48), np.float32)
    x_owns = np.empty((n_shards, 1, C1, OWN, W48), np.float32)
    s_attn = np.empty((n_shards,), np.int32)
    s_vext = np.empty((n_shards,), np.int32)
    s_dwout = np.empty((n_shards,), np.int32)
    for d in range(n_shards):
        b, half = d // 2, d % 2
        if half == 0:   # rows 0..23 owned, slab rows 0..35
            x_slabs[d, 0] = x[b, :, 0:SLAB]
            x_owns[d, 0] = x[b, :, 0:OWN]
            s_attn[d], s_vext[d], s_dwout[d] = 0, 0, 0
        else:           # rows 24..47 owned, slab rows 12..47
            x_slabs[d, 0] = x[b, :, 12:48]
            x_owns[d, 0] = x[b, :, 24:48]
            s_attn[d], s_vext[d], s_dwout[d] = 12, 9, 3
    weights = [jnp.asarray(inputs[k]) for k in _WNAMES]

    fn = _get_pmap(n_shards)
    res = fn(jnp.asarray(x_slabs), jnp.asarray(x_owns),
             jnp.asarray(s_attn), jnp.asarray(s_vext), jnp.asarray(s_dwout),
             *weights)
    res = np.asarray(jax.device_get(res))  # (8, 1, 512, 24, 48)

    out = np.empty((B, C1, 48, 48), np.float32)
    for d in range(n_shards):
        b, half = d // 2, d % 2
        out[b, :, half * OWN:(half + 1) * OWN] = res[d, 0]
    return out
